# revision 6
# baseline (speedup 1.0000x reference)
"""Trainium2 Bass kernel for batched attention (B=8, Lq=Lk=2048, D=Dv=128).

Sharding: pure data parallel - batch element b runs on NeuronCore b.

Per-core algorithm (v2 - engine-balanced restructure):

  Inputs are cast fp32->bf16 by the DMA engines (gpsimd SWDGE cast-DMA,
  with a DRAM bounce for q/k so the HWDGE xbar transpose-DMA can produce
  xqT/xkT [d, L] directly). No DVE/PE cycles are spent on input prep.

  Algebraic restructure removes both per-tensor projections on the hot path:
    scores^T = xk @ (Wk Wq^T) @ xq^T          (one fused weight W2)
      qT2 = W2 @ xq^T                          [d, Lq]  (prep, 2048 cols)
      sT_j = matmul(lhsT=xkT_j, rhs=qT2)       [128k, 1024q] per tile
    out = attn @ (xv @ Wv) = (attn @ xv) @ Wv  (defer Wv past the AV matmul)
      u = sum_j xv_j^T @ aT_j                  [d, 1024q] PSUM accum
      o_chunk = u_chunk^T @ Wv                 [128q, dv] (natural layout ->
                                                no output transposes)

  The attention mask is applied outside the exp (exp(s)*m == exp(s+bias)
  for 0/1 masks): multiplicatively into the xv stationaries (numerator)
  and inside the running-sum STT op on DVE (denominator). This frees the
  ACTIVATE from per-tile bias vectors so exp runs as 16 paired FD=2048
  ACTIVATEs over a 3-slot PSUM rotation (amortizes ACT per-instr overhead).

  softmax denominator: S_h = sum_j m_j*aT_j (DVE scalar_tensor_tensor),
  den = ones-matmul over S chunks (PE, FD=1), out = o * recip(den) (DVE).
"""

import sys

sys.path.insert(0, "/opt/trn_rl_repo")

import numpy as np

import concourse.bass as bass
import concourse.mybir as mybir
import concourse.tile as tile
from concourse import bacc
from concourse.bass_utils import run_bass_kernel_spmd
from concourse.masks import make_identity

P = 128
L = 2048
D = 128
T = L // P  # 16 k-tiles
HQ = 1024  # q-half size
F32 = mybir.dt.float32
I32 = mybir.dt.int32
BF16 = mybir.dt.bfloat16
SCALE = 1.0 / float(np.sqrt(128.0))
N_CORES = 8

ADD = mybir.AluOpType.add
MULT = mybir.AluOpType.mult
NEQ = mybir.AluOpType.not_equal
EXP = mybir.ActivationFunctionType.Exp


def build():
    nc = bacc.Bacc("TRN2", target_bir_lowering=False, debug=False)

    q_ext = nc.declare_dram_parameter("query", [L, D], F32, isOutput=False)
    k_ext = nc.declare_dram_parameter("key", [L, D], F32, isOutput=False)
    v_ext = nc.declare_dram_parameter("value", [L, D], F32, isOutput=False)
    wq_ext = nc.declare_dram_parameter("Wq", [D, D], F32, isOutput=False)
    wk_ext = nc.declare_dram_parameter("Wk", [D, D], F32, isOutput=False)
    wv_ext = nc.declare_dram_parameter("Wv", [D, D], F32, isOutput=False)
    m_ext = nc.declare_dram_parameter("mask", [1, L], I32, isOutput=False)
    out_ext = nc.declare_dram_parameter("out", [L, D], BF16, isOutput=True)

    with tile.TileContext(nc) as tc:
        with (
            tc.tile_pool(name="const", bufs=1) as const,
            tc.tile_pool(name="big", bufs=1) as big,
            tc.tile_pool(name="stage", bufs=1) as stage,
            tc.tile_pool(name="att", bufs=7) as att,
            tc.tile_pool(name="dram", bufs=1, space="DRAM") as dram,
            # score rotation: one 6-bank region sliced into 3 x [128,1024]f32
            tc.tile_pool(name="psA", bufs=1, space="PSUM") as psA,
            # 2-bank slot: warmup -> u(h) -> dps(h) -> o(h) -> u(h+1) ...
            tc.tile_pool(name="psB", bufs=1, space="PSUM") as psB,
        ):
            # ---- PE warm-up + exp-table preload while DMAs start ----
            warm = const.tile([P, P], BF16, tag="warm")
            nc.gpsimd.memset(warm[:], 0.125)

            sbig = psA.tile([P, 3 * HQ], F32, tag="sbig")
            s3 = sbig[:].rearrange("p (s q) -> p s q", s=3)

            def fillers(n):
                for _ in range(n):
                    nc.tensor.matmul(
                        s3[:, 2, 384 : 384 + P], warm[:], warm[:], start=True, stop=True
                    )

            fillers(20)
            dummy_exp = const.tile([P, 1], F32, tag="dummy")
            nc.scalar.activation(dummy_exp[:], warm[:, :1], EXP)

            # ---- input DMAs ----
            # gpsimd SWDGE cast-DMAs (fp32 -> bf16): q,k bounce via DRAM so
            # the HWDGE xbar transpose can lift them to [d, L]; v lands in
            # SBUF natural ([p, j, d] with row k = j*128+p).
            qk_dram = {
                "q": dram.tile([L, D], BF16, tag="qdram", name="qdram"),
                "k": dram.tile([L, D], BF16, tag="kdram", name="kdram"),
            }
            xv_bf = big.tile([P, T, D], BF16, tag="xv")
            v_src = v_ext[:].rearrange("(j p) d -> p j d", p=P)
            H = T // 2
            nc.gpsimd.dma_start(qk_dram["q"][0:HQ, :], q_ext[0:HQ, :])
            nc.gpsimd.dma_start(qk_dram["q"][HQ:L, :], q_ext[HQ:L, :])
            nc.gpsimd.dma_start(qk_dram["k"][0:HQ, :], k_ext[0:HQ, :])
            nc.gpsimd.dma_start(xv_bf[:, 0:H, :], v_src[:, 0:H, :])
            nc.gpsimd.dma_start(qk_dram["k"][HQ:L, :], k_ext[HQ:L, :])
            nc.gpsimd.dma_start(xv_bf[:, H:T, :], v_src[:, H:T, :])

            # sync HWDGE: weights + mask first (tiny), then xbar transposes
            wf = {}
            for name, ext in (("Wq", wq_ext), ("Wk", wk_ext), ("Wv", wv_ext)):
                wf[name] = stage.tile([P, D], F32, tag=f"wf_{name}", name=f"wf_{name}")
                nc.sync.dma_start(wf[name][:], ext[:])
            mask_i = const.tile([P, T], I32, tag="maski")
            nc.sync.dma_start(
                mask_i[:], m_ext[:].rearrange("o (j p) -> p (o j)", p=P)
            )

            xqT = big.tile([P, L], BF16, tag="xqT")
            xkT = big.tile([P, L], BF16, tag="xkT")
            for h in range(2):
                nc.sync.dma_start_transpose(
                    xqT[:, h * HQ : (h + 1) * HQ],
                    qk_dram["q"][h * HQ : (h + 1) * HQ, :],
                )
            for h in range(2):
                nc.sync.dma_start_transpose(
                    xkT[:, h * HQ : (h + 1) * HQ],
                    qk_dram["k"][h * HQ : (h + 1) * HQ, :],
                )

            # ---- weight prep on PE (psum staging carved from sbig) ----
            ident_f = stage.tile([P, P], F32, tag="identf")
            make_identity(nc, ident_f[:])
            # m01 = (mask != 0) as f32
            m01 = const.tile([P, T], F32, tag="m01")
            nc.vector.tensor_scalar(m01[:], mask_i[:], 0.0, None, NEQ)
            wv_bf = const.tile([P, D], BF16, tag="wv_bf")
            nc.vector.tensor_copy(out=wv_bf[:], in_=wf["Wv"][:])

            # WqT/WkT via fp32 transpose-mode matmuls, cast on copy-out
            wqT_ps = sbig[:, 2048 : 2048 + P]
            wkT_ps = sbig[:, 2048 + P : 2048 + 2 * P]
            nc.tensor.transpose(wqT_ps, wf["Wq"][:], ident_f[:])
            nc.tensor.transpose(wkT_ps, wf["Wk"][:], ident_f[:])
            wqT_bf = const.tile([P, D], BF16, tag="wqT")
            wkT_bf = const.tile([P, D], BF16, tag="wkT")
            nc.vector.tensor_copy(out=wqT_bf[:], in_=wqT_ps)
            nc.vector.tensor_copy(out=wkT_bf[:], in_=wkT_ps)
            fillers(6)
            # W2T = (WqT)^T @ WkT = Wq @ Wk^T  (so lhsT=W2T gives W2@x)
            w2T_ps = sbig[:, 2048 + 2 * P : 2048 + 3 * P]
            nc.tensor.matmul(w2T_ps, wqT_bf[:], wkT_bf[:], start=True, stop=True)
            w2T_bf = const.tile([P, D], BF16, tag="w2T")
            nc.vector.tensor_copy(out=w2T_bf[:], in_=w2T_ps)
            fillers(6)

            # masked value stationaries: xv_m[:, j, :] = xv * m01[k]
            xv_m = big.tile([P, T, D], BF16, tag="xv_m")
            for j in range(T):
                nc.vector.tensor_scalar_mul(
                    xv_m[:, j, :], xv_bf[:, j, :], m01[:, j : j + 1]
                )

            # qT2 = W2 @ xq^T, both halves (prep; ACT copies out while idle)
            qT2 = big.tile([P, L], BF16, tag="qT2")
            for h in range(2):
                for c in range(2):
                    nc.tensor.matmul(
                        s3[:, h, c * 512 : (c + 1) * 512],
                        w2T_bf[:],
                        xqT[:, h * HQ + c * 512 : h * HQ + (c + 1) * 512],
                        start=True,
                        stop=True,
                    )
                nc.scalar.copy(
                    out=qT2[:, h * HQ : (h + 1) * HQ], in_=s3[:, h, :]
                )
            fillers(8)

            # ---- main loop ----
            ones_col = const.tile([P, 1], BF16, tag="ones")
            nc.gpsimd.memset(ones_col[:], 1.0)
            S_h = [
                big.tile([P, HQ], BF16, tag=f"S{h}", name=f"S{h}") for h in range(2)
            ]
            out_all = big.tile([P, T, D], BF16, tag="out_all")
            out_dst = out_ext[:].rearrange("(t p) d -> p t d", p=P)

            def emit_scores(h, j, slot):
                for c in range(2):
                    nc.tensor.matmul(
                        s3[:, slot, c * 512 : (c + 1) * 512],
                        xkT[:, j * P : (j + 1) * P],
                        qT2[:, h * HQ + c * 512 : h * HQ + (c + 1) * 512],
                        start=True,
                        stop=True,
                    )

            def emit_exp(i, slots, a_pair):
                a, b = slots
                if b == a + 1:
                    src = s3[:, a : a + 2, :]
                else:
                    assert (a, b) == (2, 0)
                    src = s3[:, 2::-2, :]
                nc.scalar.activation(
                    a_pair[:].rearrange("p (s q) -> p s q", s=2),
                    src,
                    EXP,
                    scale=SCALE,
                )

            def emit_u(u_ps, j, a_pair, jj_half):
                for c in range(2):
                    nc.tensor.matmul(
                        u_ps[:, c * 512 : (c + 1) * 512],
                        xv_m[:, j, :],
                        a_pair[:, jj_half * HQ + c * 512 : jj_half * HQ + (c + 1) * 512],
                        start=(j == 0),
                        stop=(j == T - 1),
                    )

            def emit_S(h, j, a_pair, jj_half):
                a_view = a_pair[:, jj_half * HQ : (jj_half + 1) * HQ]
                if j == 0:
                    nc.vector.tensor_scalar(
                        S_h[h][:], a_view, m01[:, j : j + 1], None, MULT
                    )
                else:
                    nc.vector.scalar_tensor_tensor(
                        S_h[h][:], a_view, m01[:, j : j + 1], S_h[h][:], MULT, ADD
                    )

            # epilogue for half h, split into 4 stages so its PE/DVE bursts
            # interleave with the next half's pairs instead of stalling them
            epi_state = {}

            def epi_stageA(h, u_ps):  # DVE: evacuate u
                u_bf = big.tile([P, HQ], BF16, tag=f"u_bf{h}", name=f"u_bf{h}")
                nc.vector.tensor_copy(out=u_bf[:], in_=u_ps[:])
                epi_state[h] = {"u_bf": u_bf}

            def epi_stageB(h):  # PE: softmax denominators from S
                dps = psB.tile([P, 8], F32, tag="ub", name=f"dps{h}")
                for c in range(8):
                    nc.tensor.matmul(
                        dps[:, c : c + 1],
                        S_h[h][:, c * P : (c + 1) * P],
                        ones_col[:],
                        start=True,
                        stop=True,
                    )
                epi_state[h]["dps"] = dps

            def epi_stageC(h):  # DVE recip + PE output chunks o = u^T Wv
                dps = epi_state[h]["dps"]
                denT = const.tile([P, 8], F32, tag=f"denT{h}", name=f"denT{h}")
                nc.vector.tensor_copy(out=denT[:], in_=dps[:])
                rT = const.tile([P, 8], F32, tag=f"rT{h}", name=f"rT{h}")
                nc.vector.reciprocal(rT[:], denT[:])
                o_ps = psB.tile([P, HQ], F32, tag="ub", name=f"o{h}")
                u_bf = epi_state[h]["u_bf"]
                for c in range(8):
                    nc.tensor.matmul(
                        o_ps[:, c * P : (c + 1) * P],
                        u_bf[:, c * P : (c + 1) * P],
                        wv_bf[:],
                        start=True,
                        stop=True,
                    )
                epi_state[h].update(o_ps=o_ps, rT=rT)

            def epi_stageD(h):  # DVE scale + DMA out
                o_ps, rT = epi_state[h]["o_ps"], epi_state[h]["rT"]
                for c in range(8):
                    nc.vector.tensor_scalar_mul(
                        out_all[:, h * 8 + c, :],
                        o_ps[:, c * P : (c + 1) * P],
                        rT[:, c : c + 1],
                    )
                eng = nc.gpsimd if h == 0 else nc.sync
                for g in range(2):
                    eng.dma_start(
                        out_dst[:, h * 8 + 4 * g : h * 8 + 4 * (g + 1), :],
                        out_all[:, h * 8 + 4 * g : h * 8 + 4 * (g + 1), :],
                    )

            u_ps = {}
            pend = []  # [(h, j0, a_pair)] u-matmul work lagged behind exp

            def pop_u(n):
                for _ in range(n):
                    if not pend:
                        return
                    ph, pj, pa = pend.pop(0)
                    if ph not in u_ps:
                        u_ps[ph] = psB.tile([P, HQ], F32, tag="ub", name=f"u{ph}")
                    emit_u(u_ps[ph], pj, pa, 0)
                    emit_u(u_ps[ph], pj + 1, pa, 1)

            for h in range(2):
                for jj in range(T // 2):
                    i = h * 8 + jj
                    j0, j1 = 2 * jj, 2 * jj + 1
                    sl0, sl1 = (2 * i) % 3, (2 * i + 1) % 3
                    emit_scores(h, j0, sl0)
                    emit_scores(h, j1, sl1)
                    a_pair = att.tile([P, 2 * HQ], BF16, tag="aT", name=f"a{h}_{jj}")
                    emit_exp(i, (sl0, sl1), a_pair)
                    if h == 0:
                        pop_u(1)
                    else:
                        # h0 epilogue stages ride along the first 4 pairs;
                        # u(h1) resumes at jj=4 once the psB slot cycles back
                        if jj == 0:
                            epi_stageA(0, u_ps[0])
                        elif jj == 1:
                            epi_stageB(0)
                        elif jj == 2:
                            epi_stageC(0)
                        elif jj == 3:
                            epi_stageD(0)
                        else:
                            pop_u(2)
                    emit_S(h, j0, a_pair, 0)
                    emit_S(h, j1, a_pair, 1)
                    pend.append((h, j0, a_pair))
                if h == 0:
                    pop_u(len(pend))  # u(h0) must close before its epilogue
            pop_u(len(pend))
            epi_stageA(1, u_ps[1])
            epi_stageB(1)
            epi_stageC(1)
            epi_stageD(1)

    nc.compile()
    return nc


_NC_CACHE = None


def _get_nc():
    global _NC_CACHE
    if _NC_CACHE is None:
        _NC_CACHE = build()
    return _NC_CACHE


def kernel(query, key, value, Wq, Wk, Wv, attention_mask):
    query = np.asarray(query, dtype=np.float32)
    key = np.asarray(key, dtype=np.float32)
    value = np.asarray(value, dtype=np.float32)
    Wq = np.asarray(Wq, dtype=np.float32)
    Wk = np.asarray(Wk, dtype=np.float32)
    Wv = np.asarray(Wv, dtype=np.float32)
    mask = np.asarray(attention_mask, dtype=np.int32).reshape(N_CORES, 1, L)

    nc = _get_nc()
    in_maps = [
        {
            "query": np.ascontiguousarray(query[b]),
            "key": np.ascontiguousarray(key[b]),
            "value": np.ascontiguousarray(value[b]),
            "Wq": Wq,
            "Wk": Wk,
            "Wv": Wv,
            "mask": np.ascontiguousarray(mask[b]),
        }
        for b in range(N_CORES)
    ]
    res = run_bass_kernel_spmd(nc, in_maps, core_ids=list(range(N_CORES)))
    out = np.stack(
        [np.asarray(res.results[b]["out"]) for b in range(N_CORES)], axis=0
    )
    return out.astype(np.float32)


if __name__ == "__main__":
    rng = np.random.default_rng(0)
    q = rng.standard_normal((N_CORES, L, D), dtype=np.float32)
    k = rng.standard_normal((N_CORES, L, D), dtype=np.float32)
    v = rng.standard_normal((N_CORES, L, D), dtype=np.float32)
    wq = rng.standard_normal((128, 128), dtype=np.float32) * 0.08
    wk = rng.standard_normal((128, 128), dtype=np.float32) * 0.08
    wv = rng.standard_normal((128, 128), dtype=np.float32) * 0.08
    m = np.ones((N_CORES, 1, L), dtype=np.int32)
    out = kernel(
        query=q, key=k, value=v, Wq=wq, Wk=wk, Wv=wv, attention_mask=m
    )
    print(out.shape, out.dtype)


# revision 8
# speedup vs baseline: 1.1766x; 1.1766x over previous
"""Trainium2 Bass kernel for batched attention (B=8, Lq=Lk=2048, D=Dv=128).

Sharding: pure data parallel - batch element b runs on NeuronCore b.

Per-core algorithm (v3 - engine-balanced restructure):

  Algebraic restructure removes both per-tensor projections from the hot path:
    scores^T = xk @ (Wk Wq^T) @ xq^T          (one fused weight W2)
      qT2 = W2 @ xq^T                          [d, Lq]  (prep, 2048 cols)
      sT_j = matmul(lhsT=xkT_j, rhs=qT2)       [128k, 1024q] per tile
    out = attn @ (xv @ Wv) = (attn @ xv) @ Wv  (defer Wv past the AV matmul)
      u = sum_j xv_j^T @ aT_j                  [d, 1024q] PSUM accum
      o_chunk = u_chunk^T @ Wv                 [128q, dv] (natural layout ->
                                                no output transposes)

  Mask handling never touches the hot loop: masked k rows are zeroed in the
  xk/xv input casts (fused per-partition multiply), so masked scores are 0,
  exp gives exactly 1, and the softmax denominator is corrected by the
  constant K = #masked positions (computed once in prep):
      den_q = sum_k exp~ - K
  This frees the ACTIVATE from per-tile bias vectors, so exp runs as 16
  paired FD=2048 ACTIVATEs over a 3-slot PSUM rotation (amortizing the
  per-instruction ACT overhead), and the running sum S is a plain bf16
  tensor_tensor add (2x DVE mode).

  Input transposes run on the PE (bf16 transpose-mode matmuls -> bf16 PSUM
  -> 2x DVE copies) during the DMA window before the loop.
"""

import sys

sys.path.insert(0, "/opt/trn_rl_repo")

import numpy as np

import concourse.bass as bass
import concourse.mybir as mybir
import concourse.tile as tile
from concourse import bacc
from concourse.bass_utils import run_bass_kernel_spmd
from concourse.masks import make_identity

P = 128
L = 2048
D = 128
T = L // P  # 16 k-tiles
HQ = 1024  # q-half size
F32 = mybir.dt.float32
I32 = mybir.dt.int32
BF16 = mybir.dt.bfloat16
SCALE = 1.0 / float(np.sqrt(128.0))
N_CORES = 8

ADD = mybir.AluOpType.add
MULT = mybir.AluOpType.mult
SUB = mybir.AluOpType.subtract
NEQ = mybir.AluOpType.not_equal
EXP = mybir.ActivationFunctionType.Exp


def build():
    nc = bacc.Bacc("TRN2", target_bir_lowering=False, debug=False)

    q_ext = nc.declare_dram_parameter("query", [L, D], F32, isOutput=False)
    k_ext = nc.declare_dram_parameter("key", [L, D], F32, isOutput=False)
    v_ext = nc.declare_dram_parameter("value", [L, D], F32, isOutput=False)
    wq_ext = nc.declare_dram_parameter("Wq", [D, D], F32, isOutput=False)
    wk_ext = nc.declare_dram_parameter("Wk", [D, D], F32, isOutput=False)
    wv_ext = nc.declare_dram_parameter("Wv", [D, D], F32, isOutput=False)
    m_ext = nc.declare_dram_parameter("mask", [1, L], I32, isOutput=False)
    out_ext = nc.declare_dram_parameter("out", [L, D], BF16, isOutput=True)

    with tile.TileContext(nc) as tc:
        with (
            tc.tile_pool(name="const", bufs=1) as const,
            tc.tile_pool(name="big", bufs=1) as big,
            tc.tile_pool(name="stage", bufs=1) as stage,
            tc.tile_pool(name="att", bufs=7) as att,
            # score rotation: one 6-bank region sliced into 3 x [128,1024]f32
            tc.tile_pool(name="psA", bufs=1, space="PSUM") as psA,
            # 2-bank slot: u(h) -> dps(h) -> o(h) -> u(h+1) -> ...
            tc.tile_pool(name="psB", bufs=1, space="PSUM") as psB,
        ):
            # ---- PE warm-up + exp-table preload while DMAs start ----
            warm = const.tile([P, P], BF16, tag="warm")
            nc.gpsimd.memset(warm[:], 0.125)

            sbig = psA.tile([P, 3 * HQ], F32, tag="sbig")
            s3 = sbig[:].rearrange("p (s q) -> p s q", s=3)
            # prep-time staging carved out of rotation slot 2 (f32 cols):
            #   [2048:2432] weight transposes/products, [2432:2560] fillers,
            #   [2560:2640] mask-count pipeline
            wqT_ps = sbig[:, 2048 : 2048 + P]
            wkT_ps = sbig[:, 2048 + P : 2048 + 2 * P]
            w2T_ps = sbig[:, 2048 + 2 * P : 2048 + 3 * P]

            def fillers(n):
                for _ in range(n):
                    nc.tensor.matmul(
                        s3[:, 2, 384 : 384 + P], warm[:], warm[:],
                        start=True, stop=True,
                    )

            fillers(16)
            dummy_exp = const.tile([P, 1], F32, tag="dummy")
            nc.scalar.activation(dummy_exp[:], warm[:, :1], EXP)

            # ---- input DMAs (plain fp32 loads, two queues) ----
            # natural layout [p, t, d]: row k = p*16 + t (contiguous 8KB per
            # partition per half -> efficient DMA)
            Hh = T // 2
            xf = {}
            srcs = {}
            for name, ext in (("k", k_ext), ("q", q_ext), ("v", v_ext)):
                xf[name] = stage.tile(
                    [P, T, D], F32, tag=f"xf_{name}", name=f"xf_{name}"
                )
                srcs[name] = ext[:].rearrange("(p t) d -> p t d", p=P)
            wf = {}
            for name, ext in (("Wq", wq_ext), ("Wk", wk_ext), ("Wv", wv_ext)):
                wf[name] = stage.tile(
                    [P, D], F32, tag=f"wf_{name}", name=f"wf_{name}"
                )
                nc.sync.dma_start(wf[name][:], ext[:])
            mask_i = const.tile([P, T], I32, tag="maski")
            nc.sync.dma_start(
                mask_i[:], m_ext[:].rearrange("o (p t) -> p (o t)", p=P)
            )
            for name in ("k", "q", "v"):
                nc.sync.dma_start(xf[name][:, 0:Hh, :], srcs[name][:, 0:Hh, :])
            for name in ("k", "q", "v"):
                nc.gpsimd.dma_start(xf[name][:, Hh:T, :], srcs[name][:, Hh:T, :])

            # ---- prep: masks, weights ----
            ident_f = stage.tile([P, P], F32, tag="identf")
            make_identity(nc, ident_f[:])
            ident_bf = const.tile([P, P], BF16, tag="identbf")
            nc.vector.tensor_copy(out=ident_bf[:], in_=ident_f[:])
            m01 = const.tile([P, T], F32, tag="m01")
            nc.vector.tensor_scalar(m01[:], mask_i[:], 0.0, None, NEQ)
            wv_bf = const.tile([P, D], BF16, tag="wv_bf")
            nc.vector.tensor_copy(out=wv_bf[:], in_=wf["Wv"][:])

            nc.tensor.transpose(wqT_ps, wf["Wq"][:], ident_f[:])
            nc.tensor.transpose(wkT_ps, wf["Wk"][:], ident_f[:])
            wqT_bf = const.tile([P, D], BF16, tag="wqT")
            wkT_bf = const.tile([P, D], BF16, tag="wkT")
            nc.vector.tensor_copy(out=wqT_bf[:], in_=wqT_ps)
            nc.vector.tensor_copy(out=wkT_bf[:], in_=wkT_ps)
            fillers(4)
            # W2T = (WqT)^T @ WkT = Wq @ Wk^T  (so lhsT=W2T gives W2 @ x)
            nc.tensor.matmul(w2T_ps, wqT_bf[:], wkT_bf[:], start=True, stop=True)
            w2T_bf = const.tile([P, D], BF16, tag="w2T")
            nc.vector.tensor_copy(out=w2T_bf[:], in_=w2T_ps)

            # ---- K = #masked positions (den correction), all in prep ----
            ones_col = const.tile([P, 1], BF16, tag="ones")
            nc.gpsimd.memset(ones_col[:], 1.0)
            mneg_bf = const.tile([P, T], BF16, tag="mneg")
            # 1 - m01
            nc.vector.tensor_scalar(mneg_bf[:], m01[:], -1.0, 1.0, MULT, ADD)
            kp_ps = sbig[:, 2560:2561]
            nc.tensor.matmul(
                kp_ps[:T, :], mneg_bf[:], ones_col[:], start=True, stop=True
            )
            kp_bf = const.tile([T, 1], BF16, tag="kp")
            nc.vector.tensor_copy(out=kp_bf[:], in_=kp_ps[:T, :])
            ones16 = const.tile([T, 1], BF16, tag="ones16")
            nc.gpsimd.memset(ones16[:], 1.0)
            ksc_ps = sbig[:, 2564:2565]
            nc.tensor.matmul(
                ksc_ps[:1, :], kp_bf[:], ones16[:], start=True, stop=True
            )
            ksc_bf = const.tile([1, 1], BF16, tag="ksc")
            nc.vector.tensor_copy(out=ksc_bf[:], in_=ksc_ps[:1, :])
            ones_row = const.tile([1, P], BF16, tag="onesrow")
            nc.gpsimd.memset(ones_row[:], 1.0)
            kb_ps = sbig[:, 2568:2569]
            nc.tensor.matmul(
                kb_ps[:], ones_row[:], ksc_bf[:], start=True, stop=True
            )
            k_col = const.tile([P, 1], F32, tag="kcol")
            nc.vector.tensor_copy(out=k_col[:], in_=kb_ps[:])
            fillers(4)

            # ---- input casts (mask fused for k, v) + PE transposes ----
            # xb tile c of half h covers k rows {p*16 + (h*8+c)}
            xb = {}
            for name in ("k", "q", "v"):
                xb[name] = big.tile(
                    [P, T, D], BF16, tag=f"xb_{name}", name=f"xb_{name}"
                )
            xqT = big.tile([P, L], BF16, tag="xqT")
            xkT = big.tile([P, L], BF16, tag="xkT")
            qT2 = big.tile([P, L], BF16, tag="qT2")

            def ps_bf(slot):
                # one rotation slot viewed as [128, 2048] bf16
                return sbig[:, slot * HQ : (slot + 1) * HQ].bitcast(BF16)

            def cast_half(name, h, masked):
                if masked:
                    for c in range(Hh):
                        j = h * Hh + c
                        nc.vector.tensor_scalar(
                            xb[name][:, j, :], xf[name][:, j, :],
                            m01[:, j : j + 1], None, MULT,
                        )
                else:
                    nc.vector.tensor_copy(
                        out=xb[name][:, h * Hh : (h + 1) * Hh, :].rearrange(
                            "p a b -> p (a b)"
                        ),
                        in_=xf[name][:, h * Hh : (h + 1) * Hh, :].rearrange(
                            "p a b -> p (a b)"
                        ),
                    )

            def transpose_half(name, h, dstT, slot, part):
                # 8 transpose-mode matmuls -> bf16 psum -> one 2x DVE copy
                tview = ps_bf(slot)[:, part * HQ : (part + 1) * HQ]
                for c in range(Hh):
                    j = h * Hh + c
                    nc.tensor.matmul(
                        tview[:, c * P : (c + 1) * P],
                        xb[name][:, j, :],
                        ident_bf[:],
                        is_transpose=True,
                        start=True,
                        stop=True,
                    )
                nc.vector.tensor_copy(
                    out=dstT[:, h * HQ : (h + 1) * HQ], in_=tview
                )

            # k first (gates the loop), then q (gates qT2), v casts last
            cast_half("k", 0, True)
            transpose_half("k", 0, xkT, 0, 0)
            cast_half("q", 0, False)
            transpose_half("q", 0, xqT, 1, 0)
            cast_half("k", 1, True)
            transpose_half("k", 1, xkT, 0, 1)
            cast_half("q", 1, False)
            transpose_half("q", 1, xqT, 1, 1)
            cast_half("v", 0, True)
            cast_half("v", 1, True)
            xv_m = xb["v"]

            # qT2 = W2 @ xq^T (PE), psum -> bf16 via ACT (idle during prep)
            for h in range(2):
                for c in range(2):
                    nc.tensor.matmul(
                        s3[:, h, c * 512 : (c + 1) * 512],
                        w2T_bf[:],
                        xqT[:, h * HQ + c * 512 : h * HQ + (c + 1) * 512],
                        start=True,
                        stop=True,
                    )
                nc.scalar.copy(
                    out=qT2[:, h * HQ : (h + 1) * HQ], in_=s3[:, h, :]
                )

            # ---- main loop ----
            S_h = [
                big.tile([P, HQ], BF16, tag=f"S{h}", name=f"S{h}")
                for h in range(2)
            ]
            out_all = big.tile([P, T, D], BF16, tag="out_all")
            out_dst = out_ext[:].rearrange("(p t) d -> p t d", p=P)

            def emit_scores(h, j, slot):
                for c in range(2):
                    nc.tensor.matmul(
                        s3[:, slot, c * 512 : (c + 1) * 512],
                        xkT[:, j * P : (j + 1) * P],
                        qT2[:, h * HQ + c * 512 : h * HQ + (c + 1) * 512],
                        start=True,
                        stop=True,
                    )

            def emit_exp(slots, a_pair):
                a, b = slots
                if b == a + 1:
                    src = sbig[:, a * HQ : (a + 2) * HQ]
                    dst = a_pair[:]
                else:
                    assert (a, b) == (2, 0)
                    src = s3[:, 2::-2, :]
                    dst = a_pair[:].rearrange("p (s q) -> p s q", s=2)
                nc.scalar.activation(dst, src, EXP, scale=SCALE)

            def emit_u(u_ps, j, a_pair, jj_half):
                for c in range(2):
                    nc.tensor.matmul(
                        u_ps[:, c * 512 : (c + 1) * 512],
                        xv_m[:, j, :],
                        a_pair[
                            :, jj_half * HQ + c * 512 : jj_half * HQ + (c + 1) * 512
                        ],
                        start=(j == 0),
                        stop=(j == T - 1),
                    )

            def emit_S(h, j, a_pair, jj_half):
                a_view = a_pair[:, jj_half * HQ : (jj_half + 1) * HQ]
                if j == 0:
                    nc.vector.tensor_copy(out=S_h[h][:], in_=a_view)
                else:
                    nc.vector.tensor_tensor(S_h[h][:], S_h[h][:], a_view, ADD)

            # epilogue for half h, split into 4 stages so its PE/DVE bursts
            # interleave with the next half's pairs instead of stalling them
            epi_state = {}

            def epi_stageA(h, u_ps):  # DVE: evacuate u
                u_bf = big.tile([P, HQ], BF16, tag=f"u_bf{h}", name=f"u_bf{h}")
                nc.vector.tensor_copy(out=u_bf[:], in_=u_ps[:])
                epi_state[h] = {"u_bf": u_bf}

            def epi_stageB(h):  # PE: softmax denominators from S
                dps = psB.tile([P, 8], F32, tag="ub", name=f"dps{h}")
                for c in range(8):
                    nc.tensor.matmul(
                        dps[:, c : c + 1],
                        S_h[h][:, c * P : (c + 1) * P],
                        ones_col[:],
                        start=True,
                        stop=True,
                    )
                epi_state[h]["dps"] = dps

            def epi_stageC(h):  # DVE recip + PE output chunks o = u^T Wv
                dps = epi_state[h]["dps"]
                denT = const.tile([P, 8], F32, tag=f"denT{h}", name=f"denT{h}")
                # den = colsum(S~) - K_masked
                nc.vector.tensor_scalar(denT[:], dps[:], k_col[:], None, SUB)
                rT = const.tile([P, 8], F32, tag=f"rT{h}", name=f"rT{h}")
                nc.vector.reciprocal(rT[:], denT[:])
                o_ps = psB.tile([P, HQ], F32, tag="ub", name=f"o{h}")
                u_bf = epi_state[h]["u_bf"]
                for c in range(8):
                    nc.tensor.matmul(
                        o_ps[:, c * P : (c + 1) * P],
                        u_bf[:, c * P : (c + 1) * P],
                        wv_bf[:],
                        start=True,
                        stop=True,
                    )
                epi_state[h].update(o_ps=o_ps, rT=rT)

            def epi_stageD(h):  # DVE scale + DMA out
                o_ps, rT = epi_state[h]["o_ps"], epi_state[h]["rT"]
                for c in range(8):
                    nc.vector.tensor_scalar_mul(
                        out_all[:, h * 8 + c, :],
                        o_ps[:, c * P : (c + 1) * P],
                        rT[:, c : c + 1],
                    )
                eng = nc.gpsimd if h == 0 else nc.sync
                for g in range(2):
                    eng.dma_start(
                        out_dst[:, h * 8 + 4 * g : h * 8 + 4 * (g + 1), :],
                        out_all[:, h * 8 + 4 * g : h * 8 + 4 * (g + 1), :],
                    )

            u_ps = {}
            pend = []  # [(h, j0, a_pair)] u-matmul work lagged behind exp

            def pop_u(n):
                for _ in range(n):
                    if not pend:
                        return
                    ph, pj, pa = pend.pop(0)
                    if ph not in u_ps:
                        u_ps[ph] = psB.tile(
                            [P, HQ], F32, tag="ub", name=f"u{ph}"
                        )
                    emit_u(u_ps[ph], pj, pa, 0)
                    emit_u(u_ps[ph], pj + 1, pa, 1)

            for h in range(2):
                for jj in range(T // 2):
                    i = h * 8 + jj
                    j0, j1 = 2 * jj, 2 * jj + 1
                    sl0, sl1 = (2 * i) % 3, (2 * i + 1) % 3
                    emit_scores(h, j0, sl0)
                    emit_scores(h, j1, sl1)
                    a_pair = att.tile(
                        [P, 2 * HQ], BF16, tag="aT", name=f"a{h}_{jj}"
                    )
                    emit_exp((sl0, sl1), a_pair)
                    if h == 0:
                        pop_u(1)
                    else:
                        # h0 epilogue stages ride along the first 4 pairs;
                        # u(h1) resumes at jj=4 once the psB slot cycles back
                        if jj == 0:
                            epi_stageA(0, u_ps[0])
                        elif jj == 1:
                            epi_stageB(0)
                        elif jj == 2:
                            epi_stageC(0)
                        elif jj == 3:
                            epi_stageD(0)
                        else:
                            pop_u(2)
                    emit_S(h, j0, a_pair, 0)
                    emit_S(h, j1, a_pair, 1)
                    pend.append((h, j0, a_pair))
                if h == 0:
                    pop_u(len(pend))  # u(h0) must close before its epilogue
            pop_u(len(pend))
            epi_stageA(1, u_ps[1])
            epi_stageB(1)
            epi_stageC(1)
            epi_stageD(1)

    nc.compile()
    return nc


_NC_CACHE = None


def _get_nc():
    global _NC_CACHE
    if _NC_CACHE is None:
        _NC_CACHE = build()
    return _NC_CACHE


def kernel(query, key, value, Wq, Wk, Wv, attention_mask):
    query = np.asarray(query, dtype=np.float32)
    key = np.asarray(key, dtype=np.float32)
    value = np.asarray(value, dtype=np.float32)
    Wq = np.asarray(Wq, dtype=np.float32)
    Wk = np.asarray(Wk, dtype=np.float32)
    Wv = np.asarray(Wv, dtype=np.float32)
    mask = np.asarray(attention_mask, dtype=np.int32).reshape(N_CORES, 1, L)

    nc = _get_nc()
    in_maps = [
        {
            "query": np.ascontiguousarray(query[b]),
            "key": np.ascontiguousarray(key[b]),
            "value": np.ascontiguousarray(value[b]),
            "Wq": Wq,
            "Wk": Wk,
            "Wv": Wv,
            "mask": np.ascontiguousarray(mask[b]),
        }
        for b in range(N_CORES)
    ]
    res = run_bass_kernel_spmd(nc, in_maps, core_ids=list(range(N_CORES)))
    out = np.stack(
        [np.asarray(res.results[b]["out"]) for b in range(N_CORES)], axis=0
    )
    return out.astype(np.float32)


if __name__ == "__main__":
    rng = np.random.default_rng(0)
    q = rng.standard_normal((N_CORES, L, D), dtype=np.float32)
    k = rng.standard_normal((N_CORES, L, D), dtype=np.float32)
    v = rng.standard_normal((N_CORES, L, D), dtype=np.float32)
    wq = rng.standard_normal((128, 128), dtype=np.float32) * 0.08
    wk = rng.standard_normal((128, 128), dtype=np.float32) * 0.08
    wv = rng.standard_normal((128, 128), dtype=np.float32) * 0.08
    m = np.ones((N_CORES, 1, L), dtype=np.int32)
    out = kernel(
        query=q, key=k, value=v, Wq=wq, Wk=wk, Wv=wv, attention_mask=m
    )
    print(out.shape, out.dtype)
